# revision 78
# baseline (speedup 1.0000x reference)
"""Multi-Head Latent Attention (B=1, S=2048, HIDDEN=2048, 16 heads, MLA_DIM=128)
as a tensor-parallel Bass kernel on 8 TRN2 NeuronCores.

Sharding: 2 heads per core (q/k/v column-shard, o_proj row-shard); the
o_proj partial sums are reduced on the host.

Causal fast path: all four projections run as fp8(e4m3) DoubleRow matmuls
with a hi/lo residual split (3-term products; ~0.2% error, 0.75x the bf16
cycle cost in the TRN2 cost model, since DoubleRow packs K=256 per
instruction at 0.5 cycles/column). Tensors are pre-scaled by powers of two
to keep fp8 values out of subnormal range; the inverse scales are folded
into the rope staging copy, the exp() scale, and the o_proj evacuation.
Attention proper (scores / exp / PV) stays fp16:
  - scores are computed transposed: sT[sk, sq] with lhsT=kT tile, rhs=qT
  - exp(s*sc - 6) feeds PV directly as lhsT; an appended ones-column on V
    yields softmax denominators in the same matmuls
  - ctx rows are scaled by 1/denom (per-partition), PE-transposed, and
    split to fp8 hi/lo planes which feed the o_proj DoubleRow matmuls
    (both heads contracted per instruction via the plane dimension).
"""

import math
import os
import sys
import time

for _p in ("/opt/trn_rl_repo", "/root/.axon_site/_ro/trn_rl_repo"):
    if os.path.isdir(_p) and _p not in sys.path:
        sys.path.append(_p)

import numpy as np
import ml_dtypes

import concourse.bass as bass
import concourse.mybir as mybir
from concourse import bacc
from concourse.masks import make_identity
from concourse.tile import TileContext

B, S, HID = 1, 2048, 2048
NUM_HEADS, MLA = 16, 128
HEAD_DIM = HID // NUM_HEADS  # 128
ROPE_BASE = 10000.0
N_CORES = 8
HPC = NUM_HEADS // N_CORES  # heads per core = 2
DL = HPC * MLA              # local projection width = 256
P = 128
KT = HID // P               # 16 contraction tiles
KP = KT // 2                # 8 contraction k-pairs (DoubleRow)
ST = S // P                 # 16 sequence tiles
SCALE = 1.0 / math.sqrt(MLA)
VA = MLA + 1                # v columns + ones column = 129
FP = mybir.dt.float32
BF = mybir.dt.float16  # 16-bit compute dtype (fp16: better mantissa)
F8 = mybir.dt.float8e4
F8NP = ml_dtypes.float8_e4m3
NEG = -1e9
EXP_BIAS = -6.0  # exp(s-6): cancels in softmax, keeps fp16 in range
DR = mybir.MatmulPerfMode.DoubleRow

# fp8 pre-scales (powers of two; inverses folded downstream)
A_H = 32.0     # hidden
A_W = 1024.0   # wq/wk/wv
A_C = 32.0     # normalized ctx
A_WO = 1024.0  # wo
RAW_SC = 2.0 ** -8              # rope staging copy scale (q,k at 2^7)
EXP_SC = SCALE / (2.0 ** 14)    # scores psum is at scale (A_H*A_W*RAW_SC)^2
V_SC = 1.0 / (A_H * A_W)        # v evacuation scale
OPROJ_SC = 1.0 / (A_C * A_WO)   # o_proj evacuation scale


# --------------------------------------------------------------------------
# causal fast path
# --------------------------------------------------------------------------

def _emit_causal(nc, tc, aps):
    hhi_d, hlo_d, wqk_d = aps["hhi"], aps["hlo"], aps["wqk"]
    wvhi_d, wvlo_d = aps["wvhi"], aps["wvlo"]
    wohi_d, wolo_d = aps["wohi"], aps["wolo"]
    cosT_d, sinT_d, out = aps["cosT"], aps["sinT"], aps["out"]
    H2 = MLA // 2

    with (
        tc.tile_pool(name="persist", bufs=1) as pp,
        tc.tile_pool(name="work", bufs=1) as wp,
    ):
        qT = [pp.tile([P, S], BF, name=f"qT{h}") for h in range(HPC)]
        kT = [pp.tile([P, S], BF, name=f"kT{h}") for h in range(HPC)]
        # v (+ ones col): [sk-part, s-tile, head, VA]
        vaug = pp.tile([P, ST, HPC, VA], BF, name="vaug")
        nc.gpsimd.memset(vaug[:, :, :, MLA:VA], 1.0)
        ctxhi = pp.tile([P, HPC, S], F8, name="ctxhi")
        ctxlo = pp.tile([P, HPC, S], F8, name="ctxlo")
        wohi = pp.tile([P, HPC, HID], F8, name="wohi")
        wolo = pp.tile([P, HPC, HID], F8, name="wolo")
        ident = pp.tile([P, P], BF, name="ident")
        make_identity(nc, ident)
        ebias = pp.tile([P, 1], FP, name="ebias")
        nc.gpsimd.memset(ebias, EXP_BIAS)

        def rope_raw(sp, w, raw_eng, bufs=6):
            """psum chunk -> fp16 staging at 2^-8 scale (frees the bank)."""
            raw = wp.tile([P, w], BF, tag="raw", bufs=5, name="raw")
            if raw_eng == "act":
                nc.scalar.activation(raw, sp,
                                     mybir.ActivationFunctionType.Copy,
                                     scale=RAW_SC)
            else:
                nc.vector.tensor_scalar_mul(raw, sp, RAW_SC)
            return raw

        def rope_dve(raw, dstT, c0, w):
            cols = slice(c0, c0 + w)
            swp = wp.tile([P, w], BF, tag="swp", bufs=2, name="swp")
            nc.vector.tensor_copy(swp[0:H2, :], raw[H2:P, :])
            nc.vector.tensor_copy(swp[H2:P, :], raw[0:H2, :])
            nc.vector.tensor_mul(swp, swp, sin_sb[:, cols])
            nc.vector.tensor_mul(dstT[:, cols], raw, cos_sb[:, cols])
            nc.vector.tensor_add(dstT[:, cols], dstT[:, cols], swp)

        def rope_chunk(sp, dstT, c0, w, raw_eng):
            """psum chunk [P, w] (scale 2^15) -> roped fp16 in dstT[:, c0:c0+w]
            (scale 2^7). sin_sb is sign-folded on the host (rows 0:64)."""
            rope_dve(rope_raw(sp, w, raw_eng), dstT, c0, w)

        with tc.tile_pool(name="hp", bufs=1) as hp:
            # ---- P1: stream h + w in k-pairs; q0/k0 proj accumulates ----
            wpair = [hp.tile([P, 4, 2, DL], F8, name=f"wp{k}")
                     for k in range(KP)]
            hph = [hp.tile([P, 2, S], F8, name=f"hph{k}") for k in range(KP)]
            hpl = [hp.tile([P, 2, S], F8, name=f"hpl{k}") for k in range(KP)]
            wvhi = hp.tile([P, KT, DL], F8, name="wvhi")
            wvlo = hp.tile([P, KT, DL], F8, name="wvlo")
            cos_sb = hp.tile([P, S], BF, name="cos_sb")
            sin_sb = hp.tile([P, S], BF, name="sin_sb")

            def hpl_src(kp):
                return hlo_d[:, 2 * kp * S:(2 * kp + 2) * S]

            # One DMA queue, issue order == consumption order.  (DMAs on
            # other queues would issue immediately and jump ahead of the h
            # stream on the shared DMA engines.)
            for kp in range(KP):
                if kp == 0:
                    nc.sync.dma_start(wpair[0][:, 0:2, :, :],
                                      wqk_d[:, 0:1024])
                else:
                    nc.sync.dma_start(wpair[kp],
                                      wqk_d[:, kp * 4 * 2 * DL:
                                            (kp + 1) * 4 * 2 * DL])
                hs = slice(2 * kp * S, (2 * kp + 2) * S)
                if kp == 0:
                    # halves: the first chunks' matmuls can start sooner;
                    # first two pieces ride the scalar queue to issue in
                    # parallel with wpair0 on the sync queue
                    nc.scalar.dma_start(hph[0][:, 0, 0:1024],
                                        hhi_d[:, 0:1024])
                    nc.scalar.dma_start(hph[0][:, 1, 0:1024],
                                        hhi_d[:, S:S + 1024])
                    for dst, dsrc in ((hph[0], hhi_d), (hpl[0], hlo_d)):
                        for half in range(2):
                            if dst is hph[0] and half == 0:
                                continue
                            cs = slice(half * 1024, half * 1024 + 1024)
                            nc.sync.dma_start(dst[:, 0, cs], dsrc[:, cs])
                            nc.sync.dma_start(
                                dst[:, 1, cs],
                                dsrc[:, S + half * 1024:S + half * 1024
                                     + 1024])
                        if dst is hph[0]:
                            nc.sync.dma_start(wpair[0][:, 2:4, :, :],
                                              wqk_d[:, 1024:2048])
                else:
                    nc.sync.dma_start(hph[kp], hhi_d[:, hs])
                    nc.sync.dma_start(hpl[kp], hpl_src(kp))
            # rope tables in 512-col chunks, matching rope consumption order
            for c in range(4):
                cc = slice(c * 512, (c + 1) * 512)
                nc.sync.dma_start(cos_sb[:, cc], cosT_d[:, cc])
                nc.sync.dma_start(sin_sb[:, cc], sinT_d[:, cc])
            nc.sync.dma_start(wvhi, wvhi_d[:, :])
            nc.sync.dma_start(wvlo, wvlo_d[:, :])
            nc.sync.dma_start(wohi, wohi_d[:, :])
            nc.sync.dma_start(wolo, wolo_d[:, :])

            # ---- attention generator (one head) ----
            def scores_t(h, t, e, pq):
                c0 = t // 4
                start_col = t * P
                for cc in range(c0, 4):
                    lo = max(cc * 512, start_col)
                    w = (cc + 1) * 512 - lo
                    sp = pq.tile([P, 512], FP, tag="sc", bufs=2, name="scp")
                    nc.tensor.matmul(sp[:, 0:w],
                                     lhsT=kT[h][:, t * P:(t + 1) * P],
                                     rhs=qT[h][:, lo:lo + w],
                                     start=True, stop=True)
                    nc.scalar.activation(
                        e[:, lo - start_col:lo - start_col + w], sp[:, 0:w],
                        mybir.ActivationFunctionType.Exp,
                        bias=ebias[:, 0:1], scale=EXP_SC)
                    if cc == c0:
                        # causal mask of the diagonal block, post-exp (SBUF
                        # in/out so Pool can do it): keep sk <= sq
                        nc.gpsimd.affine_select(
                            out=e[:, 0:P], in_=e[:, 0:P],
                            compare_op=mybir.AluOpType.is_ge,
                            fill=0.0, base=0, pattern=[[1, P]],
                            channel_multiplier=-1)

            def attn_gen(h, lag, ep, pq, oproj_cb=None, pre_tiles=(),
                         hi_eng="act"):
                expT = list(pre_tiles)
                pre = len(expT)

                def pv_m(m):
                    ctx = pq.tile([P, VA], FP, tag="ctx", bufs=2, name="ctx")
                    for k in range(m + 1):
                        nc.tensor.matmul(
                            ctx, lhsT=expT[k][:, (m - k) * P:(m - k + 1) * P],
                            rhs=vaug[:, k, h, 0:VA],
                            start=(k == 0), stop=(k == m))
                    rc = wp.tile([P, 1], FP, tag="rc", bufs=4, name="rc")
                    nc.vector.reciprocal(rc, ctx[:, MLA:VA])
                    ctxn = wp.tile([P, P], BF, tag="cn", bufs=3, name="cn")
                    nc.vector.tensor_scalar_mul(ctxn, ctx[:, 0:MLA], rc)
                    tp = pq.tile([P, P + 1], BF, tag="tp", bufs=1, name="tp")
                    nc.tensor.transpose(tp[:, 0:P], ctxn, ident)
                    mcols = slice(m * P, (m + 1) * P)
                    if hi_eng == "act":
                        nc.scalar.activation(
                            ctxhi[:, h, mcols], tp[:, 0:P],
                            mybir.ActivationFunctionType.Copy, scale=A_C)
                        lo = nc.vector
                    else:
                        nc.vector.tensor_scalar_mul(ctxhi[:, h, mcols],
                                                    tp[:, 0:P], A_C)
                        lo = nc.gpsimd
                    lo.scalar_tensor_tensor(
                        ctxlo[:, h, mcols], tp[:, 0:P], A_C,
                        ctxhi[:, h, mcols],
                        op0=mybir.AluOpType.mult,
                        op1=mybir.AluOpType.subtract)

                next_pv = 0
                for t in range(pre, ST):
                    e = ep.tile([P, S - t * P], BF, name=f"e{h}_{t}")
                    expT.append(e)
                    scores_t(h, t, e, pq)
                    yield
                    while next_pv <= t - lag:
                        m = next_pv
                        next_pv += 1
                        pv_m(m)
                        if oproj_cb is not None and m >= 1:
                            oproj_cb(m - 1, m - 1 >= ST - 2)
                        yield
                while next_pv < ST:
                    m = next_pv
                    next_pv += 1
                    pv_m(m)
                    if oproj_cb is not None and m >= 1:
                        oproj_cb(m - 1, m - 1 >= ST - 2)
                    yield
                if oproj_cb is not None:
                    oproj_cb(ST - 1, True)

            # ---- o_proj for one sq-tile (both heads per DR instruction) ----
            ev_rr = [0]

            def make_oproj(pq):
                return lambda m, final: oproj_m(m, pq, final)

            def oproj_m(m, pq, final):
                ob = wp.tile([P, HID], BF, tag="ob", bufs=3, name="ob")
                mcols = slice(m * P, (m + 1) * P)
                nq = 4
                cw = HID // nq
                for q4 in range(nq):
                    op = pq.tile([P, 512], FP, tag="op", bufs=3, name="op")
                    cols = slice(q4 * cw, (q4 + 1) * cw)
                    nc.tensor.matmul(op[:, 0:cw], lhsT=ctxhi[:, 0:2, mcols],
                                     rhs=wohi[:, 0:2, cols],
                                     start=True, stop=False, perf_mode=DR)
                    nc.tensor.matmul(op[:, 0:cw], lhsT=ctxhi[:, 0:2, mcols],
                                     rhs=wolo[:, 0:2, cols],
                                     start=False, stop=False, perf_mode=DR)
                    nc.tensor.matmul(op[:, 0:cw], lhsT=ctxlo[:, 0:2, mcols],
                                     rhs=wohi[:, 0:2, cols],
                                     start=False, stop=True, perf_mode=DR)
                    if final:
                        act = q4 % 2 == 1
                    else:
                        act = (m >= 2
                               and (ev_rr[0] % 16) in (1, 3, 5, 7, 9, 11, 13))
                        ev_rr[0] += 1
                    if act:
                        nc.scalar.activation(
                            ob[:, cols], op[:, 0:cw],
                            mybir.ActivationFunctionType.Copy, scale=OPROJ_SC)
                    else:
                        nc.vector.tensor_scalar_mul(ob[:, cols], op[:, 0:cw],
                                                    OPROJ_SC)
                    if final and q4 == 1:
                        nc.sync.dma_start(out[m * P:(m + 1) * P, 0:1024],
                                          ob[:, 0:1024])
                if final:
                    nc.sync.dma_start(out[m * P:(m + 1) * P, 1024:2048],
                                      ob[:, 1024:2048])
                else:
                    nc.sync.dma_start(out[m * P:(m + 1) * P, :], ob)

            # ---- P2 filler: q1/k1 proj, v proj, then early h1 scores ----
            def proj2_gen(pq, deferred):
                for raw, rdstT, rc0 in deferred:
                    rope_dve(raw, rdstT, rc0, 512)
                    yield
                for widx, dstT, hb0, crange in (
                        (0, qT[1], 1, (0, 1, 2, 3)),
                        (1, kT[1], 1, (0, 1, 2, 3))):
                    for c in crange:
                        sp = pq.tile([P, 512], FP, tag="pj", bufs=3,
                                     name="pjp")
                        cols = slice(c * 512, (c + 1) * 512)
                        hb = slice(hb0 * P, (hb0 + 1) * P)
                        for kp in range(KP):
                            nc.tensor.matmul(
                                sp, lhsT=wpair[kp][:, widx, 0:2, hb],
                                rhs=hph[kp][:, 0:2, cols],
                                start=(kp == 0), stop=False, perf_mode=DR)
                            nc.tensor.matmul(
                                sp, lhsT=wpair[kp][:, 2 + widx, 0:2, hb],
                                rhs=hph[kp][:, 0:2, cols],
                                start=False, stop=False, perf_mode=DR)
                            nc.tensor.matmul(
                                sp, lhsT=wpair[kp][:, widx, 0:2, hb],
                                rhs=hpl[kp][:, 0:2, cols],
                                start=False, stop=(kp == KP - 1),
                                perf_mode=DR)
                        rope_chunk(sp, dstT, c * 512, 512,
                                   raw_eng=("act" if widx == 0 and c < 2
                                            else "dve"))
                        yield
                for b in list(range(5)) + ["pre"] + list(range(5, ST)):
                    if b == "pre":
                        # early h1 score tiles: fills the P2->P3 transition
                        for t in range(N_PRE):
                            scores_t(1, t, e1pre[t], pq)
                            yield
                        continue
                    sp = pq.tile([P, 512], FP, tag="pj", bufs=3,
                                 name="vbp")
                    bb = slice(b * P, (b + 1) * P)
                    for kp in range(KP):
                        wv2 = slice(2 * kp, 2 * kp + 2)
                        nc.tensor.matmul(sp[:, 0:2 * MLA],
                                         lhsT=hph[kp][:, 0:2, bb],
                                         rhs=wvhi[:, wv2, :],
                                         start=(kp == 0), stop=False,
                                         perf_mode=DR)
                        nc.tensor.matmul(sp[:, 0:2 * MLA],
                                         lhsT=hpl[kp][:, 0:2, bb],
                                         rhs=wvhi[:, wv2, :],
                                         start=False, stop=False,
                                         perf_mode=DR)
                        nc.tensor.matmul(sp[:, 0:2 * MLA],
                                         lhsT=hph[kp][:, 0:2, bb],
                                         rhs=wvlo[:, wv2, :],
                                         start=False, stop=(kp == KP - 1),
                                         perf_mode=DR)
                    nc.vector.tensor_scalar_mul(vaug[:, b, 0:2, 0:MLA],
                                                sp[:, 0:2 * MLA], V_SC)
                    yield

            def interleave(ga, gb, ratio=1):
                alive_a, alive_b = True, True
                while alive_a or alive_b:
                    if alive_a:
                        try:
                            next(ga)
                        except StopIteration:
                            alive_a = False
                    for _ in range(ratio):
                        if alive_b:
                            try:
                                next(gb)
                            except StopIteration:
                                alive_b = False

            # ---- P1: 8 psum chunks q0 c0..c3, k0 c0..c3 (all 8 banks) ----
            # 4-term products (incl. lo*lo) here: the extra work keeps PE
            # paced with the h DMA stream so the p-state ramp never resets.
            with tc.tile_pool(name="pq1", bufs=1, space="PSUM") as pq1:
                p1ps = [pq1.tile([P, 512], FP, tag="p1", bufs=8,
                                 name=f"p1_{i}") for i in range(8)]
                p1chunks = [(p1ps[i], i // 4, i % 4) for i in range(8)]

                def p1_mm(ps, lhs, rhs, cols, start, stop):
                    nc.tensor.matmul(ps, lhsT=lhs, rhs=rhs[:, 0:2, cols],
                                     start=start, stop=stop, perf_mode=DR)

                deferred = []

                def p1_mm(ps, lhs, rhs, cols, start, stop):
                    nc.tensor.matmul(ps, lhsT=lhs, rhs=rhs[:, 0:2, cols],
                                     start=start, stop=stop, perf_mode=DR)

                # Chunk i holds back pair i, emitted after the stream ends:
                # chunk closings stagger, so the psum-pool close barrier
                # (which waits on every chunk's raw-copy) clears just as the
                # last matmul retires instead of 8 serial raws later.
                for kp in range(KP):
                    lolo = kp < KP - 2  # 3-term on last two pairs
                    for i, (ps, widx, c) in enumerate(p1chunks):
                        if kp == i % 2:
                            continue
                        cols = slice(c * 512, (c + 1) * 512)
                        start = kp == (1 if i % 2 == 0 else 0)
                        p1_mm(ps, wpair[kp][:, widx, 0:2, 0:P], hph[kp],
                              cols, start, False)
                        p1_mm(ps, wpair[kp][:, 2 + widx, 0:2, 0:P], hph[kp],
                              cols, False, False)
                    for i, (ps, widx, c) in enumerate(p1chunks):
                        if kp == i % 2:
                            continue
                        cols = slice(c * 512, (c + 1) * 512)
                        p1_mm(ps, wpair[kp][:, widx, 0:2, 0:P], hpl[kp],
                              cols, False, False)
                        if lolo:
                            p1_mm(ps, wpair[kp][:, 2 + widx, 0:2, 0:P],
                                  hpl[kp], cols, False, False)
                for i, (ps, widx, c) in enumerate(p1chunks):
                    kp = i % 2
                    lolo = True
                    cols = slice(c * 512, (c + 1) * 512)
                    p1_mm(ps, wpair[kp][:, widx, 0:2, 0:P], hph[kp],
                          cols, False, False)
                    p1_mm(ps, wpair[kp][:, 2 + widx, 0:2, 0:P], hph[kp],
                          cols, False, False)
                    p1_mm(ps, wpair[kp][:, widx, 0:2, 0:P], hpl[kp],
                          cols, False, not lolo)
                    if lolo:
                        p1_mm(ps, wpair[kp][:, 2 + widx, 0:2, 0:P], hpl[kp],
                              cols, False, True)
                    dstT = qT[0] if widx == 0 else kT[0]
                    if i >= 5:  # k0 c1..c3: free the bank now, rope later
                        deferred.append((rope_raw(ps, 512, "act"),
                                         dstT, c * 512))
                    else:
                        rope_chunk(ps, dstT, c * 512, 512, raw_eng="act")

            with (
                tc.tile_pool(name="ep0", bufs=1) as ep0,
                tc.tile_pool(name="pq2", bufs=1, space="PSUM") as pq2,
            ):
                pg = proj2_gen(pq2, deferred)
                # rope units (no PE work) + q1 chunks as PE backlog
                for _ in range(5):
                    next(pg)
                interleave(attn_gen(0, 8, ep0, pq2), pg, ratio=1)

        # ---- P3: attn h1 + o_proj + out DMA (h pool freed) ----
        with (
            tc.tile_pool(name="ep1", bufs=1) as ep1,
            tc.tile_pool(name="pq3", bufs=1, space="PSUM") as pq3,
        ):
            for _ in attn_gen(1, 2, ep1, pq3, oproj_cb=make_oproj(pq3),
                              pre_tiles=e1pre, hi_eng="dve"):
                pass


def _rope_tables():
    inv = (1.0 / (ROPE_BASE ** (np.arange(0, MLA, 2, dtype=np.float32) / MLA)))
    t = np.arange(S, dtype=np.float32)
    freqs = np.outer(t, inv).astype(np.float32)          # [S, 64]
    emb = np.concatenate([freqs, freqs], axis=-1)        # [S, 128]
    cosT = np.ascontiguousarray(np.cos(emb).astype(np.float32).T)
    sinT = np.ascontiguousarray(np.sin(emb).astype(np.float32).T)
    sinT[0:MLA // 2, :] *= -1.0  # sign-fold for the swap-halves rope form
    return cosT, sinT


def _split8(x, alpha):
    xs = x * np.float32(alpha)
    hi = xs.astype(F8NP)
    lo = (xs - hi.astype(np.float32)).astype(F8NP)
    return hi, lo


def _make_in_maps_causal(hidden, wq, wk, wv, wo):
    hT = np.ascontiguousarray(hidden.reshape(S, HID).T)   # [HID, S]
    h_s = hT.reshape(KT, P, S).transpose(1, 0, 2).reshape(P, KT * S)
    hhi, hlo = _split8(h_s, A_H)
    cosT, sinT = _rope_tables()
    cosT = cosT.astype(np.float16)
    sinT = sinT.astype(np.float16)

    maps = []
    for c in range(N_CORES):
        sl = slice(c * DL, (c + 1) * DL)

        def tile_w(w):
            # [HID, DL] -> [P, KT, DL]
            return w[:, sl].reshape(KT, P, DL).transpose(1, 0, 2)

        wq_hi, wq_lo = _split8(tile_w(wq), A_W)
        wk_hi, wk_lo = _split8(tile_w(wk), A_W)

        def pairs(a):
            # [P, KT, DL] -> [P, KP, 1, 2, DL]
            return a.reshape(P, KP, 1, 2, DL)

        wqk = np.concatenate(
            [pairs(wq_hi), pairs(wk_hi), pairs(wq_lo), pairs(wk_lo)],
            axis=2).reshape(P, KP * 4 * 2 * DL)
        wv_hi, wv_lo = _split8(tile_w(wv), A_W)
        wo_s = wo[sl, :].reshape(HPC, P, HID).transpose(1, 0, 2)
        wo_hi, wo_lo = _split8(wo_s, A_WO)
        maps.append({
            "hhi": hhi, "hlo": hlo,
            "wqk": np.ascontiguousarray(wqk),
            "wvhi": np.ascontiguousarray(wv_hi.reshape(P, KT * DL)),
            "wvlo": np.ascontiguousarray(wv_lo.reshape(P, KT * DL)),
            "wohi": np.ascontiguousarray(wo_hi.reshape(P, HPC * HID)),
            "wolo": np.ascontiguousarray(wo_lo.reshape(P, HPC * HID)),
            "cosT": cosT, "sinT": sinT,
        })
    return maps


# --------------------------------------------------------------------------
# full / mask fallback (original bf16 kernel)
# --------------------------------------------------------------------------

def _emit(nc, tc, aps, variant):
    """Emit the per-core program. variant in ("full", "mask")."""
    hT, wq, wk, wv, wo, cosT, sinT, out = (
        aps["hT"], aps["wq"], aps["wk"], aps["wv"], aps["wo"],
        aps["cosT"], aps["sinT"], aps["out"])
    maskT = aps.get("maskT")

    with (
        tc.tile_pool(name="psum", bufs=1, space="PSUM") as pq,
        tc.tile_pool(name="persist", bufs=1) as pp,
        tc.tile_pool(name="work", bufs=1) as wp,
        tc.tile_pool(name="wstream", bufs=1) as ws,
    ):
        qT = [pp.tile([P, S], BF, name=f"qT{h}") for h in range(HPC)]
        kT = [pp.tile([P, S], BF, name=f"kT{h}") for h in range(HPC)]
        vaug = pp.tile([P, ST * HPC * VA], BF, name="vaug")
        ctxT = [pp.tile([P, S], BF, name=f"ctxT{h}") for h in range(HPC)]
        ident = pp.tile([P, P], BF, name="ident")
        make_identity(nc, ident)
        for t in range(ST):
            for h in range(HPC):
                nc.gpsimd.memset(vaug[:, t * HPC * VA + h * VA + MLA:
                                      t * HPC * VA + h * VA + VA], 1.0)
        ebias = pp.tile([P, 1], FP, name="ebias")
        nc.gpsimd.memset(ebias, EXP_BIAS)

        def load_wm(wdram, m):
            wt = ws.tile([P, KT * P], BF, tag="wm", bufs=3, name="wm")
            nc.sync.dma_start(wt, wdram[m * P:(m + 1) * P, :])
            return wt

        def rope(state, sp, dst, c0, w):
            cos_sb, sin_sb = state["cos_sb"], state["sin_sb"]
            cols = slice(c0, c0 + w)
            raw = wp.tile([P, w], BF, tag="qraw", bufs=3, name="raw")
            nc.scalar.copy(raw, sp)
            swp = wp.tile([P, w], BF, tag="tmpb", bufs=3, name="ropeswp")
            H2 = MLA // 2
            nc.vector.tensor_copy(swp[0:H2, :], raw[H2:P, :])
            nc.vector.tensor_copy(swp[H2:P, :], raw[0:H2, :])
            nc.vector.tensor_mul(swp, swp, sin_sb[:, cols])
            nc.vector.tensor_mul(dst[:, cols], raw, cos_sb[:, cols])
            nc.vector.tensor_add(dst[:, cols], dst[:, cols], swp)

        def mm_chunk(state, wt, c0, tag, bufs, w):
            ht_sb = state["ht_sb"]
            sp = pq.tile([P, w], FP, tag=tag, bufs=bufs, name="mmps")
            for k in range(KT):
                for c in range(w // 512):
                    nc.tensor.matmul(
                        sp[:, c * 512:(c + 1) * 512],
                        lhsT=wt[:, k * P:(k + 1) * P],
                        rhs=ht_sb[k][:, c0 + c * 512:c0 + (c + 1) * 512],
                        start=(k == 0), stop=(k == KT - 1))
            return sp

        def rope_tensor_gen(state, wsrc, h, dst, tag, bufs, w, wm_pre=None):
            wm = wm_pre if wm_pre is not None else load_wm(wsrc, h)
            for cc in range(S // w):
                sp = mm_chunk(state, wm, cc * w, tag, bufs, w)
                rope(state, sp, dst, cc * w, w)
                yield

        def v_tensor_gen(state, h, tag, bufs, w, wm_pre=None, cc0=0):
            wm = wm_pre if wm_pre is not None else load_wm(wv, h)
            for cc in range(cc0, S // w):
                sp = mm_chunk(state, wm, cc * w, tag, bufs, w)
                vt = wp.tile([P, w], BF, tag="tmpf", bufs=2, name="vtmp")
                nc.vector.tensor_copy(vt, sp)
                yield
                for b in range(w // P):
                    t = (cc * w) // P + b
                    tp = pq.tile([P, P + 1], BF, tag="small", bufs=2,
                                 name="vtp")
                    nc.tensor.transpose(tp[:, 0:P], vt[:, b * P:(b + 1) * P],
                                        ident)
                    nc.vector.tensor_copy(
                        vaug[:, t * HPC * VA + h * VA:
                             t * HPC * VA + h * VA + MLA], tp[:, 0:P])
                    yield

        def chain(*gens):
            for g in gens:
                yield from g

        def head_proj_gen(state, h, tag, bufs, w, wmq_pre=None):
            return chain(
                rope_tensor_gen(state, wq, h, qT[h], tag, bufs, w,
                                wm_pre=wmq_pre),
                rope_tensor_gen(state, wk, h, kT[h], tag, bufs, w),
                v_tensor_gen(state, h, tag, bufs, w))

        def head_attn_gen(h, after_tile=None, ep=None, lag=1):
            ep = ep if ep is not None else wp
            expT = []

            def emit_scores(t):
                e = ep.tile([P, S], BF, tag="expTw", bufs=ST, name=f"ew{t}")
                expT.append((e, 0))
                for cc in range(2):
                    cols = slice(cc * 1024, (cc + 1) * 1024)
                    sp = pq.tile([P, 1024], FP, tag="b2", bufs=2, name="scps")
                    for c in range(2):
                        nc.tensor.matmul(
                            sp[:, c * 512:(c + 1) * 512],
                            lhsT=kT[h][:, t * P:(t + 1) * P],
                            rhs=qT[h][:, cc * 1024 + c * 512:
                                      cc * 1024 + (c + 1) * 512],
                            start=True, stop=True)
                    if maskT is not None:
                        mt = ep.tile([P, 1024], FP, tag="mt", bufs=4,
                                     name="mt")
                        nc.sync.dma_start(
                            mt, maskT[t * P:(t + 1) * P, cols])
                        nc.vector.tensor_add(sp, sp, mt)
                    nc.scalar.activation(
                        e[:, cc * 1024:(cc + 1) * 1024], sp,
                        mybir.ActivationFunctionType.Exp, bias=ebias[:, 0:1],
                        scale=SCALE)

            def finish_tile(m):
                ctx = pq.tile([P, VA], FP, tag="small", bufs=2, name="ctx")
                ks = list(range(ST))
                for k in ks:
                    ek, ekoff = expT[k]
                    nc.tensor.matmul(
                        ctx[:, 0:VA],
                        lhsT=ek[:, m * P - ekoff:(m + 1) * P - ekoff],
                        rhs=vaug[:, k * HPC * VA + h * VA:
                                 k * HPC * VA + h * VA + VA],
                        start=(k == ks[0]), stop=(k == ks[-1]))
                recip = wp.tile([P, 1], FP, tag="recip", bufs=4, name="rc")
                nc.vector.reciprocal(recip, ctx[:, MLA:VA])
                ctxn = wp.tile([P, P], BF, tag="ctxn", bufs=4, name="cn")
                nc.vector.tensor_scalar_mul(ctxn, ctx[:, 0:MLA], recip)
                tp = pq.tile([P, P + 1], BF, tag="small", bufs=2, name="ctp")
                nc.tensor.transpose(tp[:, 0:P], ctxn, ident)
                nc.vector.tensor_copy(ctxT[h][:, m * P:(m + 1) * P],
                                      tp[:, 0:P])
                if after_tile is not None:
                    after_tile(m)

            for t in range(ST):
                emit_scores(t)
                yield
            for m in range(ST):
                finish_tile(m)
                yield

        def make_oproj():
            wo_sb = []
            for h in range(HPC):
                wt = wp.tile([P, HID], BF, tag="wosb", bufs=2, name=f"wo{h}")
                nc.sync.dma_start(wt, wo[h * P:(h + 1) * P, :])
                wo_sb.append(wt)

            def oproj_m(m):
                for q4 in range(4):
                    op = pq.tile([P, 512], FP, tag="b2p", bufs=2, name="ops")
                    for h in range(HPC):
                        nc.tensor.matmul(
                            op, lhsT=ctxT[h][:, m * P:(m + 1) * P],
                            rhs=wo_sb[h][:, q4 * 512:(q4 + 1) * 512],
                            start=(h == 0), stop=(h == HPC - 1))
                    ob = wp.tile([P, 512], BF, tag="ob", bufs=6, name="ob")
                    act_share = 2 if m >= ST - 6 else 1
                    if q4 < act_share:
                        nc.scalar.copy(ob, op)
                    else:
                        nc.vector.tensor_copy(ob, op)
                    nc.sync.dma_start(
                        out[m * P:(m + 1) * P, q4 * 512:(q4 + 1) * 512], ob)
            return oproj_m

        def load_resident(hp):
            state = {}
            ht_sb = []
            for k in range(KT):
                ht = hp.tile([P, S], BF, name=f"ht{k}")
                eng = nc.scalar if k % 2 == 0 else nc.sync
                eng.dma_start(ht, hT[k * P:(k + 1) * P, :])
                ht_sb.append(ht)
            state["ht_sb"] = ht_sb
            cos_sb = hp.tile([P, S], BF, name="cos_sb")
            sin_sb = hp.tile([P, S], BF, name="sin_sb")
            nc.scalar.dma_start(cos_sb, cosT[:, :])
            nc.scalar.dma_start(sin_sb, sinT[:, :])
            state["cos_sb"] = cos_sb
            state["sin_sb"] = sin_sb
            return state

        def run(gen):
            for _ in gen:
                pass

        with tc.tile_pool(name="htp", bufs=1) as hp:
            wmq0 = load_wm(wq, 0)
            state = load_resident(hp)
            run(head_proj_gen(state, 0, "b2", 2, 1024, wmq_pre=wmq0))
            run(head_proj_gen(state, 1, "b2", 2, 1024))
        with tc.tile_pool(name="expp", bufs=1) as ep:
            run(head_attn_gen(0, ep=ep))
            oproj_m = make_oproj()
            run(head_attn_gen(1, after_tile=oproj_m, ep=ep))


def _build(variant):
    nc = bacc.Bacc("TRN2", target_bir_lowering=False, debug=False,
                   enable_asserts=False, num_devices=N_CORES)
    if variant == "causal":
        aps = {
            "hhi": nc.dram_tensor("hhi", [P, KT * S], F8,
                                  kind="ExternalInput").ap(),
            "hlo": nc.dram_tensor("hlo", [P, KT * S], F8,
                                  kind="ExternalInput").ap(),
            "wqk": nc.dram_tensor("wqk", [P, KP * 4 * 2 * DL], F8,
                                  kind="ExternalInput").ap(),
            "wvhi": nc.dram_tensor("wvhi", [P, KT * DL], F8,
                                   kind="ExternalInput").ap(),
            "wvlo": nc.dram_tensor("wvlo", [P, KT * DL], F8,
                                   kind="ExternalInput").ap(),
            "wohi": nc.dram_tensor("wohi", [P, HPC * HID], F8,
                                   kind="ExternalInput").ap(),
            "wolo": nc.dram_tensor("wolo", [P, HPC * HID], F8,
                                   kind="ExternalInput").ap(),
            "cosT": nc.dram_tensor("cosT", [MLA, S], BF,
                                   kind="ExternalInput").ap(),
            "sinT": nc.dram_tensor("sinT", [MLA, S], BF,
                                   kind="ExternalInput").ap(),
            "out": nc.dram_tensor("out", [S, HID], BF,
                                  kind="ExternalOutput").ap(),
        }
        with TileContext(nc) as tc:
            _emit_causal(nc, tc, aps)
    else:
        aps = {
            "hT": nc.dram_tensor("hT", [HID, S], BF,
                                 kind="ExternalInput").ap(),
            "wq": nc.dram_tensor("wq", [DL, KT * P], BF,
                                 kind="ExternalInput").ap(),
            "wk": nc.dram_tensor("wk", [DL, KT * P], BF,
                                 kind="ExternalInput").ap(),
            "wv": nc.dram_tensor("wv", [DL, KT * P], BF,
                                 kind="ExternalInput").ap(),
            "wo": nc.dram_tensor("wo", [DL, HID], BF,
                                 kind="ExternalInput").ap(),
            "cosT": nc.dram_tensor("cosT", [MLA, S], BF,
                                   kind="ExternalInput").ap(),
            "sinT": nc.dram_tensor("sinT", [MLA, S], BF,
                                   kind="ExternalInput").ap(),
            "out": nc.dram_tensor("out", [S, HID], BF,
                                  kind="ExternalOutput").ap(),
        }
        if variant == "mask":
            aps["maskT"] = nc.dram_tensor("maskT", [S, S], FP,
                                          kind="ExternalInput").ap()
        with TileContext(nc) as tc:
            _emit(nc, tc, aps, variant)
    nc.compile()
    return nc


_CAUSAL_REF = None


def _detect_variant(mask2d):
    global _CAUSAL_REF
    if not mask2d.any():
        return "full"
    if _CAUSAL_REF is None:
        _CAUSAL_REF = np.where(
            np.tril(np.ones((S, S), dtype=bool)), np.float32(0.0),
            np.float32(NEG)).astype(np.float32)
    if np.array_equal(mask2d, _CAUSAL_REF):
        return "causal"
    return "mask"


def _make_in_maps(hidden, wq, wk, wv, wo, mask2d, variant):
    if variant == "causal":
        return _make_in_maps_causal(hidden, wq, wk, wv, wo)
    bf = np.float16
    hTn = np.ascontiguousarray(hidden.reshape(S, HID).T).astype(bf)
    cosT, sinT = _rope_tables()
    cosT, sinT = cosT.astype(bf), sinT.astype(bf)
    wqb, wkb, wvb = wq.astype(bf), wk.astype(bf), wv.astype(bf)
    wob = wo.astype(bf)

    def pretile(w, c):
        ws_ = w[:, c * DL:(c + 1) * DL]
        return np.ascontiguousarray(
            ws_.reshape(KT, P, HPC, P).transpose(2, 1, 0, 3).reshape(
                HPC * P, KT * P))

    maps = []
    for c in range(N_CORES):
        m = {
            "hT": hTn,
            "wq": pretile(wqb, c),
            "wk": pretile(wkb, c),
            "wv": pretile(wvb, c),
            "wo": np.ascontiguousarray(wob[c * DL:(c + 1) * DL, :]),
            "cosT": cosT,
            "sinT": sinT,
        }
        if variant == "mask":
            m["maskT"] = np.ascontiguousarray(mask2d.T) * np.float32(1.0 / SCALE)
        maps.append(m)
    return maps


class Runner:
    """Compiled program + reusable jitted sharded executable."""

    def __init__(self, variant):
        self.variant = variant
        self.nc = _build(variant)
        self._jit = None
        self._meta = None

    def _prep(self):
        import jax
        from jax.sharding import Mesh, NamedSharding, PartitionSpec
        from jax.experimental.shard_map import shard_map
        from concourse import bass2jax
        from concourse.bass2jax import _bass_exec_p, install_neuronx_cc_hook

        from concourse.bass2jax import partition_id_tensor

        install_neuronx_cc_hook()
        nc = self.nc
        part_name = (nc.partition_id_tensor.name
                     if nc.partition_id_tensor else None)
        in_names, out_names, out_avals = [], [], []
        for alloc in nc.m.functions[0].allocations:
            if not isinstance(alloc, mybir.MemoryLocationSet):
                continue
            name = alloc.memorylocations[0].name
            if alloc.kind == "ExternalInput":
                if name != part_name:
                    in_names.append(name)
            elif alloc.kind == "ExternalOutput":
                out_names.append(name)
                out_avals.append(jax.core.ShapedArray(
                    tuple(alloc.tensor_shape), mybir.dt.np(alloc.dtype)))
        n_params = len(in_names)
        all_names = in_names + out_names
        if part_name is not None:
            all_names = all_names + [part_name]

        def _body(*args):
            operands = list(args)
            if part_name is not None:
                operands.append(partition_id_tensor())
            outs = _bass_exec_p.bind(
                *operands, out_avals=tuple(out_avals),
                in_names=tuple(all_names),
                out_names=tuple(out_names), lowering_input_output_aliases=(),
                sim_require_finite=True, sim_require_nnan=True, nc=nc)
            return tuple(outs)

        devices = jax.devices()[:N_CORES]
        mesh = Mesh(np.asarray(devices), ("core",))
        nsh = NamedSharding(mesh, PartitionSpec("core"))
        n_outs = len(out_names)
        jitted = jax.jit(
            shard_map(_body, mesh=mesh,
                      in_specs=(PartitionSpec("core"),) * (n_params + n_outs),
                      out_specs=(PartitionSpec("core"),) * n_outs,
                      check_rep=False),
            donate_argnums=tuple(range(n_params, n_params + n_outs)),
            keep_unused=True)
        self._jit = jitted
        self._meta = (in_names, out_names, out_avals, nsh)

    def run(self, in_maps):
        """One execution; returns list of per-core output dicts."""
        import jax
        if self._jit is None:
            self._prep()
        in_names, out_names, out_avals, nsh = self._meta
        concat_in = [
            jax.device_put(
                np.concatenate([m[n] for m in in_maps], axis=0), nsh)
            for n in in_names]
        zeros = [
            jax.device_put(
                np.zeros((N_CORES * a.shape[0], *a.shape[1:]), a.dtype), nsh)
            for a in out_avals]
        outs = self._jit(*concat_in, *zeros)
        outs = [np.asarray(o) for o in outs]
        return [
            {n: outs[i].reshape(N_CORES, *out_avals[i].shape)[c]
             for i, n in enumerate(out_names)}
            for c in range(N_CORES)]

    def time_exec(self, in_maps, iters=20):
        """Median wall-clock seconds per on-device execution."""
        import jax
        if self._jit is None:
            self._prep()
        in_names, out_names, out_avals, nsh = self._meta
        concat_in = [
            jax.device_put(
                np.concatenate([m[n] for m in in_maps], axis=0), nsh)
            for n in in_names]
        zero_np = [np.zeros((N_CORES * a.shape[0], *a.shape[1:]), a.dtype)
                   for a in out_avals]
        out = self._jit(*concat_in, *[jax.device_put(z, nsh) for z in zero_np])
        jax.block_until_ready(out)
        times = []
        for _ in range(iters):
            zs = [jax.device_put(z, nsh) for z in zero_np]
            jax.block_until_ready(zs)
            t0 = time.perf_counter()
            out = self._jit(*concat_in, *zs)
            jax.block_until_ready(out)
            times.append(time.perf_counter() - t0)
        return float(np.median(times))


_RUNNERS = {}


def _get_runner(variant):
    if variant not in _RUNNERS:
        _RUNNERS[variant] = Runner(variant)
    return _RUNNERS[variant]


def kernel(hidden_states, wq, wk, wv, wo, attention_mask):
    hidden_states = np.asarray(hidden_states, dtype=np.float32)
    wq = np.asarray(wq, dtype=np.float32)
    wk = np.asarray(wk, dtype=np.float32)
    wv = np.asarray(wv, dtype=np.float32)
    wo = np.asarray(wo, dtype=np.float32)
    mask2d = np.asarray(attention_mask, dtype=np.float32)[0, 0]
    assert hidden_states.shape == (B, S, HID)

    variant = _detect_variant(mask2d)
    runner = _get_runner(variant)
    in_maps = _make_in_maps(hidden_states, wq, wk, wv, wo, mask2d, variant)
    results = runner.run(in_maps)
    acc = np.zeros((S, HID), dtype=np.float64)
    for c in range(N_CORES):
        acc += results[c]["out"]
    return acc.astype(np.float32).reshape(B, S, HID)
